# revision 64
# baseline (speedup 1.0000x reference)
import sys

sys.path.insert(0, "/opt/trn_rl_repo")

import numpy as np

import concourse.bass as bass
import concourse.tile as tile
from concourse import mybir
from concourse.masks import make_identity
from concourse.vector_clock import ScopedClock

B, C_LEN, Q_LEN, H = 16, 2048, 128, 128
N_CORES = 8
B_PER_CORE = B // N_CORES
NT = C_LEN // 128
NG = 4
GS = NT // NG
F32 = mybir.dt.float32
BF16 = mybir.dt.bfloat16
I32 = mybir.dt.int32
AX = mybir.AxisListType.X
EXP = mybir.ActivationFunctionType.Exp
IDENT = mybir.ActivationFunctionType.Identity

MAX_WAITS_PER_INST = 1


def _split_excess_waits(nc, insts):
    out = []
    for inst in insts:
        si = getattr(inst, "sync_info", None)
        waits = list(si.on_wait) if si is not None and si.on_wait else []
        if len(waits) > MAX_WAITS_PER_INST and type(inst).__name__.startswith("Inst"):
            extra = waits[: -MAX_WAITS_PER_INST or None]
            keep = waits[-MAX_WAITS_PER_INST:]
            for i in range(0, len(extra), MAX_WAITS_PER_INST):
                out.append(
                    mybir.InstNoOp(
                        name=nc.get_next_instruction_name(),
                        sync_info=mybir.SyncInfo(
                            on_wait=extra[i : i + MAX_WAITS_PER_INST], on_update=[]
                        ),
                        bass_nofuse=True,
                        engine=inst.engine,
                    )
                )
            inst.sync_info = mybir.SyncInfo(
                on_wait=keep, on_update=list(si.on_update or [])
            )
        out.append(inst)
    return out


class SplitDrainTileContext(tile.TileContext):

    def _lower_ordered_insts(self, ordered):
        for bb_name in list(ordered.keys()):
            ordered[bb_name] = _split_excess_waits(self.nc, ordered[bb_name])
        return super()._lower_ordered_insts(ordered)

    def _drain_and_barrier(self, tick_clock, wait_clock):
        nc = self.nc
        drain_inst = nc.sync.drain()
        wait_clock.add_sem_waits(
            drain_inst.ins, ScopedClock({None: tick_clock.global_clock})
        )
        si = drain_inst.ins.sync_info
        waits = list(si.on_wait) if si is not None and si.on_wait else []
        if waits:
            drain_inst.ins.sync_info = mybir.SyncInfo(
                on_wait=[], on_update=list(si.on_update or [])
            )
            engs = [nc.sync, nc.vector, nc.scalar, nc.tensor, nc.gpsimd]
            for j, i in enumerate(range(0, len(waits), MAX_WAITS_PER_INST)):
                nop = engs[j % len(engs)].nop()
                nop.ins.sync_info = mybir.SyncInfo(
                    on_wait=waits[i : i + MAX_WAITS_PER_INST], on_update=[]
                )
        nc.all_engine_barrier()
        assert self.sems is not None
        popped = nc._tile_sem_poison_stack.pop()
        assert popped is self._sem_poison
        nc.clear_and_free_semaphores(list(self.sems.allocated().values()))
        nc.all_engine_barrier()


def build_nc() -> bass.Bass:
    nc = bass.Bass()
    ctxT_d = nc.dram_tensor("ctxT", [B_PER_CORE, H, C_LEN], BF16, kind="ExternalInput")
    ctx_d = nc.dram_tensor("ctx", [B_PER_CORE, 128, NT, H], BF16, kind="ExternalInput")
    QW = 256 + NT + 1
    qmix_d = nc.dram_tensor(
        "qmix", [B_PER_CORE, 128, QW + 3], BF16, kind="ExternalInput"
    )
    g23_d = nc.dram_tensor("G23", [B_PER_CORE, 128, NT, 256], BF16, kind="ExternalOutput")
    g4_d = nc.dram_tensor("G4", [B_PER_CORE, 128, NT * 128], BF16, kind="ExternalOutput")

    from contextlib import ExitStack

    with SplitDrainTileContext(nc) as tc, ExitStack() as es:
        consts = es.enter_context(tc.tile_pool(name="consts", bufs=1))
        bp = es.enter_context(tc.tile_pool(name="bp", bufs=2))
        psT = es.enter_context(tc.tile_pool(name="psT", bufs=2, space="PSUM"))
        psTT = es.enter_context(tc.tile_pool(name="psTT", bufs=2, space="PSUM"))
        psCQ = es.enter_context(tc.tile_pool(name="psCQ", bufs=2, space="PSUM"))
        psM = es.enter_context(tc.tile_pool(name="psM", bufs=2, space="PSUM"))

        identity = consts.tile([128, 128], F32)
        make_identity(nc, identity)
        ones_row_bf = consts.tile([1, 128], BF16)
        nc.gpsimd.memset(ones_row_bf, 1.0)
        ones_col_bf = consts.tile([128, 1], BF16)
        nc.gpsimd.memset(ones_col_bf, 1.0)
        wcq_f32 = consts.tile([128, 1], F32)
        neg6e4 = consts.tile([128, 1], F32)
        nc.gpsimd.memset(neg6e4, -60000.0)


        def emit_load_head(b):
            L = {}
            qmix_sb = bp.tile(
                [128, QW + 3], BF16, tag="qmix", name=f"qmix_{b}"
            )
            nc.sync.dma_start(out=qmix_sb, in_=qmix_d[b])
            L.update(qryT=qmix_sb[:, 0:128], qry=qmix_sb[:, 128:256],
                     cmT=qmix_sb[:, 256 : 256 + NT],
                     qm=qmix_sb[:, 256 + NT : 256 + NT + 1],
                     wc=qmix_sb[:, QW : QW + 1], wq=qmix_sb[:, QW + 1 : QW + 2],
                     wcq=qmix_sb[:, QW + 2 : QW + 3])
            return L

        def emit_load_ctxT_piece(b, L, lo, hi):
            if "ctxT" not in L:
                L["ctxT"] = bp.tile(
                    [128, C_LEN], BF16, tag="ctxT", name=f"ctxT_{b}"
                )
            nc.sync.dma_start(out=L["ctxT"][:, lo:hi], in_=ctxT_d[b][:, lo:hi])

        def emit_load_ctx(b, L):
            ctx_sb = bp.tile([128, NT, H], BF16, tag="ctx", name=f"ctx_{b}")
            nc.sync.dma_start(out=ctx_sb, in_=ctx_d[b])
            L["ctx"] = ctx_sb

        def emit_prelims_a(b, L):
            if b == 0:
                nc.vector.tensor_copy(out=wcq_f32, in_=L["wcq"])
            qTw = bp.tile([128, 128], BF16, tag="qTw", name=f"qTw_{b}")
            nc.vector.tensor_scalar_mul(qTw, L["qryT"], wcq_f32)

            ps_misc = psM.tile([128, 512], F32, tag="misc")
            L["ps_sc"] = ps_misc[:, 0:16]
            L["ps_z"] = ps_misc[:, 16:32]
            L["ps_q2c"] = ps_misc[:, 32:33]
            L["ps_sr"] = ps_misc[0:1, 34:162]
            L["ps_qr"] = ps_misc[0:1, 34:162]
            L["ps_zb"] = ps_misc[0:1, 162:178]

            ps_sq = ps_misc[:, 33:34]
            nc.tensor.matmul(ps_sq, L["qryT"], L["wq"], start=True, stop=True)
            qoff = bp.tile([128, 1], F32, tag="qoff", name=f"qoff_{b}")
            nc.scalar.activation(
                out=qoff, in_=L["qm"], func=IDENT, scale=60000.0, bias=neg6e4
            )
            sqm_col = bp.tile([128, 1], F32, tag="sqm", name=f"sqm_{b}")
            nc.scalar.add(out=sqm_col, in_=ps_sq, add=qoff)
            eT = bp.tile([128, C_LEN], BF16, tag="eT", name=f"eT_{b}")
            L.update(qTw=qTw, sqm=sqm_col, eT=eT)

        def emit_prelims_b(b, L):
            nc.tensor.transpose(L["ps_sr"], L["sqm"], identity)
            srow = bp.tile([1, 128], BF16, tag="srow", name=f"srow_{b}")
            nc.vector.tensor_copy(out=srow, in_=L["ps_sr"])

            cmoff = bp.tile([128, NT], F32, tag="cmoff", name=f"cmoff_{b}")
            nc.gpsimd.tensor_copy(out=cmoff, in_=L["cmT"])
            nc.gpsimd.tensor_scalar(
                out=cmoff, in0=cmoff, scalar1=1.0, scalar2=60000.0,
                op0=mybir.AluOpType.subtract, op1=mybir.AluOpType.mult,
            )

            m_buf = bp.tile([128, NT], F32, tag="m_buf", name=f"m_buf_{b}")
            u_buf = bp.tile([128, NT], F32, tag="u_buf", name=f"u_buf_{b}")
            e_b = bp.tile([128, NT], BF16, tag="e_b", name=f"e_b_{b}")
            gbuf = bp.tile([128, NT, 256], BF16, tag="gbuf", name=f"gbuf_{b}")
            g4buf = bp.tile([128, NT, 128], BF16, tag="g4buf", name=f"g4buf_{b}")
            L.update(srow=srow, cmoff=cmoff,
                     m_buf=m_buf, u_buf=u_buf, e_b=e_b,
                     gbuf=gbuf, g4buf=g4buf)

        def emit_tt(b, L, g):
            c0 = g * GS * 128
            ps_tt = psTT.tile([128, GS * 128], F32, tag="TT")
            nc.tensor.matmul(
                ps_tt, L["qTw"], L["ctxT"][:, c0 : c0 + GS * 128],
                start=True, stop=True,
            )
            eT_g = L["eT"][:, c0 : c0 + GS * 128]
            nc.scalar.activation(out=eT_g, in_=ps_tt, func=EXP, bias=L["sqm"])

        def emit_phase1(b, L, g, halves=False):
            qry, ctx, gbuf = L["qry"], L["ctx"], L["gbuf"]
            ps_cq = psCQ.tile(
                [128, GS, 128], F32, tag="CQ", name=f"CQ_{b}_{g}"
            )
            dr = bp.tile([128, GS], F32, tag=f"dr{b}_{g % 2}", name=f"dr{b}_{g}")
            waves = ((0, 2), (2, 4)) if halves else ((0, 4),)
            for lo, hi in waves:
                ws = slice(g * GS + lo, g * GS + hi)
                for i in range(lo, hi):
                    t = g * GS + i
                    eT_t = L["eT"][:, t * 128 : (t + 1) * 128]
                    nc.tensor.matmul(
                        ps_cq[:, i], eT_t, qry, start=True, stop=True
                    )
                    nc.tensor.matmul(
                        L["ps_z"][:, t : t + 1], eT_t, ones_col_bf,
                        start=True, stop=True,
                    )
                nc.vector.reciprocal(out=dr[:, lo:hi], in_=L["ps_z"][:, ws])
                if g % 2 == 0:
                    w = hi - lo
                    dr_b = dr[:, lo:hi].unsqueeze(2).broadcast_to([128, w, 128])
                    nc.vector.tensor_mul(
                        out=gbuf[:, ws, 0:128], in0=ps_cq[:, lo:hi], in1=dr_b
                    )
                else:
                    for i in range(lo, hi):
                        t = g * GS + i
                        nc.scalar.activation(
                            out=gbuf[:, t, 0:128], in_=ps_cq[:, i], func=IDENT,
                            scale=dr[:, i : i + 1],
                        )
                eng = nc.gpsimd if g % 2 == 0 else nc.vector
                eng.tensor_mul(
                    out=gbuf[:, ws, 128:256], in0=ctx[:, ws],
                    in1=gbuf[:, ws, 0:128],
                )
                nc.sync.dma_start(out=g23_d[b][:, ws], in_=gbuf[:, ws])

        def emit_phase2(b, L, g):
            ctxT, qTw, srow = L["ctxT"], L["qTw"], L["srow"]
            ps = slice(g * GS, (g + 1) * GS)
            ps_T = psT.tile([128, GS, 128], F32, tag="T")
            for i in range(GS):
                t = g * GS + i
                ctxT_t = ctxT[:, t * 128 : (t + 1) * 128]
                nc.tensor.matmul(ps_T[:, i], ctxT_t, qTw, start=True, stop=False)
                nc.tensor.matmul(ps_T[:, i], ones_row_bf, srow, start=False, stop=True)
                nc.tensor.matmul(
                    L["ps_sc"][:, t : t + 1], ctxT_t, L["wc"], start=True, stop=True
                )
            nc.vector.reduce_max(out=L["m_buf"][:, ps], in_=ps_T, axis=AX)
            u_g = L["u_buf"][:, ps]
            nc.vector.tensor_add(out=u_g, in0=L["ps_sc"][:, ps], in1=L["m_buf"][:, ps])
            nc.vector.tensor_add(out=u_g, in0=u_g, in1=L["cmoff"][:, ps])
            e_g = L["e_b"][:, ps]
            nc.scalar.activation(out=e_g, in_=u_g, func=EXP)
            nc.tensor.matmul(
                L["ps_zb"][:, ps], ones_col_bf, e_g, start=True, stop=True
            )

        def emit_q2c_mms(b, L):
            for t in range(NT):
                nc.tensor.matmul(
                    L["ps_q2c"], L["ctx"][:, t], L["e_b"][:, t : t + 1],
                    start=(t == 0), stop=(t == NT - 1),
                )

        def emit_tail(b, L):
            ctx, g4buf, e_b = L["ctx"], L["g4buf"], L["e_b"]
            z_tot = bp.tile([1, 1], F32, tag="z_tot")
            nc.vector.reduce_sum(out=z_tot, in_=L["ps_zb"], axis=AX)
            zr = bp.tile([1, 1], F32, tag="zr")
            nc.vector.reciprocal(out=zr, in_=z_tot)

            q2c_col = bp.tile([128, 1], F32, tag="q2c_col")
            nc.scalar.copy(out=q2c_col, in_=L["ps_q2c"])
            nc.tensor.transpose(L["ps_qr"], q2c_col, identity)
            q2c_row = bp.tile([1, 128], BF16, tag="q2c_row")
            nc.vector.tensor_scalar_mul(q2c_row, L["ps_qr"], zr)

            ps_bc = psTT.tile([128, GS * 128], F32, tag="TT", name=f"bc_{b}")
            nc.tensor.matmul(
                ps_bc[:, 0:128], ones_row_bf, q2c_row, start=True, stop=True
            )
            bc_sb = bp.tile([128, 128], BF16, tag="bc_sb")
            nc.scalar.copy(out=bc_sb, in_=ps_bc[:, 0:128])
            bc4 = bc_sb.unsqueeze(1).broadcast_to([128, GS, 128])
            g4flat = g4buf.rearrange("p t h -> p (t h)")
            nc.vector.tensor_mul(out=g4buf[:, 0:4], in0=ctx[:, 0:4], in1=bc4)
            nc.gpsimd.tensor_mul(out=g4buf[:, 4:8], in0=ctx[:, 4:8], in1=bc4)
            nc.sync.dma_start(out=g4_d[b][:, 0:1024], in_=g4flat[:, 0:1024])
            nc.vector.tensor_mul(out=g4buf[:, 8:12], in0=ctx[:, 8:12], in1=bc4)
            nc.sync.dma_start(out=g4_d[b][:, 1024:1536], in_=g4flat[:, 1024:1536])
            nc.vector.tensor_mul(
                out=g4buf[:, 12:16], in0=ctx[:, 12:16], in1=bc4
            )
            nc.sync.dma_start(out=g4_d[b][:, 1536:2048], in_=g4flat[:, 1536:2048])

        Ls = [None] * B_PER_CORE
        Ls[0] = emit_load_head(0)
        emit_load_ctxT_piece(0, Ls[0], 0, 512)
        emit_load_ctxT_piece(0, Ls[0], 512, 1024)
        Ls[1] = emit_load_head(1)
        emit_load_ctxT_piece(0, Ls[0], 1024, 2048)
        emit_load_ctxT_piece(1, Ls[1], 0, 1024)
        emit_load_ctx(0, Ls[0])
        emit_load_ctxT_piece(1, Ls[1], 1024, 2048)
        emit_load_ctx(1, Ls[1])

        with tc.high_priority():
            emit_prelims_a(0, Ls[0])
            emit_prelims_a(1, Ls[1])
            emit_tt(0, Ls[0], 0)
            emit_tt(1, Ls[1], 0)
        emit_prelims_b(0, Ls[0])
        emit_prelims_b(1, Ls[1])
        for g in range(NG):
            if g + 1 < NG:
                emit_tt(0, Ls[0], g + 1)
            emit_phase1(0, Ls[0], g)
            emit_phase2(0, Ls[0], g)
            if g + 1 < NG:
                emit_tt(1, Ls[1], g + 1)
            emit_phase2(1, Ls[1], g)
            emit_phase1(1, Ls[1], g)
        emit_q2c_mms(0, Ls[0])
        emit_q2c_mms(1, Ls[1])
        emit_tail(0, Ls[0])
        emit_tail(1, Ls[1])

    return nc


_NC_CACHE = None


def _get_nc():
    global _NC_CACHE
    if _NC_CACHE is None:
        _NC_CACHE = build_nc()
    return _NC_CACHE


def _prep_core_inputs(context, query, W, context_mask, query_mask, sl):
    import ml_dtypes

    BF = ml_dtypes.bfloat16
    ctx = np.asarray(context[sl], dtype=np.float32)
    qry = np.asarray(query[sl], dtype=np.float32)
    ctx_bf = ctx.astype(BF)
    qry_bf = qry.astype(BF)
    n = ctx.shape[0]
    cmT = (
        np.asarray(context_mask[sl], dtype=np.float32)
        .reshape(n, NT, 128)
        .transpose(0, 2, 1)
        .astype(BF)
    )
    qm = np.asarray(query_mask[sl], dtype=np.float32).astype(BF)[:, :, None]
    w3 = np.stack(
        [
            np.asarray(W[:H], dtype=np.float32),
            np.asarray(W[H : 2 * H], dtype=np.float32),
            np.asarray(W[2 * H :], dtype=np.float32),
        ],
        axis=1,
    ).astype(BF)
    w3b = np.broadcast_to(w3[None], (n, H, 3))
    qmix = np.concatenate(
        [qry_bf.transpose(0, 2, 1), qry_bf, cmT, qm, w3b], axis=2
    )
    return {
        "ctxT": np.ascontiguousarray(ctx_bf.transpose(0, 2, 1)),
        "ctx": np.ascontiguousarray(
            ctx_bf.reshape(n, NT, 128, H).transpose(0, 2, 1, 3)
        ),
        "qmix": np.ascontiguousarray(qmix),
    }


def kernel(context, query, W, context_mask, query_mask):
    from concourse.bass_utils import run_bass_kernel_spmd

    context = np.ascontiguousarray(np.asarray(context, dtype=np.float32))
    query = np.ascontiguousarray(np.asarray(query, dtype=np.float32))
    W = np.ascontiguousarray(np.asarray(W, dtype=np.float32))
    context_mask = np.ascontiguousarray(np.asarray(context_mask, dtype=np.int32))
    query_mask = np.ascontiguousarray(np.asarray(query_mask, dtype=np.int32))

    nc = _get_nc()
    in_maps = []
    for c in range(N_CORES):
        sl = slice(c * B_PER_CORE, (c + 1) * B_PER_CORE)
        in_maps.append(
            _prep_core_inputs(context, query, W, context_mask, query_mask, sl)
        )
    res = run_bass_kernel_spmd(nc, in_maps, core_ids=list(range(N_CORES)))

    out = np.empty((B, C_LEN, 4 * H), dtype=np.float32)
    out[:, :, 0:128] = context
    for c in range(N_CORES):
        sl = slice(c * B_PER_CORE, (c + 1) * B_PER_CORE)
        g23 = np.asarray(res.results[c]["G23"]).astype(np.float32)
        g4 = np.asarray(res.results[c]["G4"]).astype(np.float32)
        out[sl, :, 128:384] = g23.transpose(0, 2, 1, 3).reshape(
            B_PER_CORE, C_LEN, 256
        )
        out[sl, :, 384:512] = (
            g4.reshape(B_PER_CORE, 128, NT, 128)
            .transpose(0, 2, 1, 3)
            .reshape(B_PER_CORE, C_LEN, 128)
        )
    return out


if __name__ == "__main__":
    from concourse.timeline_sim import TimelineSim

    nc = build_nc()
    dur = TimelineSim(nc).simulate()
    print(f"TimelineSim estimated duration: {dur:.0f} ns")


# revision 66
# speedup vs baseline: 1.0136x; 1.0136x over previous
import sys

sys.path.insert(0, "/opt/trn_rl_repo")

import numpy as np

import concourse.bass as bass
import concourse.tile as tile
from concourse import mybir
from concourse.masks import make_identity
from concourse.vector_clock import ScopedClock

B, C_LEN, Q_LEN, H = 16, 2048, 128, 128
N_CORES = 8
B_PER_CORE = B // N_CORES
NT = C_LEN // 128
NG = 4
GS = NT // NG
F32 = mybir.dt.float32
BF16 = mybir.dt.bfloat16
I32 = mybir.dt.int32
AX = mybir.AxisListType.X
EXP = mybir.ActivationFunctionType.Exp
IDENT = mybir.ActivationFunctionType.Identity

MAX_WAITS_PER_INST = 1


def _split_excess_waits(nc, insts):
    out = []
    for inst in insts:
        si = getattr(inst, "sync_info", None)
        waits = list(si.on_wait) if si is not None and si.on_wait else []
        if len(waits) > MAX_WAITS_PER_INST and type(inst).__name__.startswith("Inst"):
            extra = waits[: -MAX_WAITS_PER_INST or None]
            keep = waits[-MAX_WAITS_PER_INST:]
            for i in range(0, len(extra), MAX_WAITS_PER_INST):
                out.append(
                    mybir.InstNoOp(
                        name=nc.get_next_instruction_name(),
                        sync_info=mybir.SyncInfo(
                            on_wait=extra[i : i + MAX_WAITS_PER_INST], on_update=[]
                        ),
                        bass_nofuse=True,
                        engine=inst.engine,
                    )
                )
            inst.sync_info = mybir.SyncInfo(
                on_wait=keep, on_update=list(si.on_update or [])
            )
        out.append(inst)
    return out


class SplitDrainTileContext(tile.TileContext):

    def _lower_ordered_insts(self, ordered):
        for bb_name in list(ordered.keys()):
            ordered[bb_name] = _split_excess_waits(self.nc, ordered[bb_name])
        return super()._lower_ordered_insts(ordered)

    def _drain_and_barrier(self, tick_clock, wait_clock):
        nc = self.nc
        drain_inst = nc.sync.drain()
        wait_clock.add_sem_waits(
            drain_inst.ins, ScopedClock({None: tick_clock.global_clock})
        )
        si = drain_inst.ins.sync_info
        waits = list(si.on_wait) if si is not None and si.on_wait else []
        if waits:
            drain_inst.ins.sync_info = mybir.SyncInfo(
                on_wait=[], on_update=list(si.on_update or [])
            )
            engs = [nc.sync, nc.vector, nc.scalar, nc.tensor, nc.gpsimd]
            for j, i in enumerate(range(0, len(waits), MAX_WAITS_PER_INST)):
                nop = engs[j % len(engs)].nop()
                nop.ins.sync_info = mybir.SyncInfo(
                    on_wait=waits[i : i + MAX_WAITS_PER_INST], on_update=[]
                )
        nc.all_engine_barrier()
        assert self.sems is not None
        popped = nc._tile_sem_poison_stack.pop()
        assert popped is self._sem_poison
        nc.clear_and_free_semaphores(list(self.sems.allocated().values()))
        nc.all_engine_barrier()


def build_nc() -> bass.Bass:
    nc = bass.Bass()
    ctxT_d = nc.dram_tensor("ctxT", [B_PER_CORE, H, C_LEN], BF16, kind="ExternalInput")
    ctx_d = nc.dram_tensor("ctx", [B_PER_CORE, 128, NT, H], BF16, kind="ExternalInput")
    QW = 256 + NT + 1
    qmix_d = nc.dram_tensor(
        "qmix", [B_PER_CORE, 128, QW + 3], BF16, kind="ExternalInput"
    )
    g23_d = nc.dram_tensor("G23", [B_PER_CORE, 128, NT, 256], BF16, kind="ExternalOutput")
    g4_d = nc.dram_tensor("G4", [B_PER_CORE, 128, NT * 128], BF16, kind="ExternalOutput")

    from contextlib import ExitStack

    with SplitDrainTileContext(nc) as tc, ExitStack() as es:
        consts = es.enter_context(tc.tile_pool(name="consts", bufs=1))
        bp = es.enter_context(tc.tile_pool(name="bp", bufs=2))
        psT = es.enter_context(tc.tile_pool(name="psT", bufs=2, space="PSUM"))
        psTT = es.enter_context(tc.tile_pool(name="psTT", bufs=2, space="PSUM"))
        psCQ = es.enter_context(tc.tile_pool(name="psCQ", bufs=2, space="PSUM"))
        psM = es.enter_context(tc.tile_pool(name="psM", bufs=2, space="PSUM"))

        identity = consts.tile([128, 128], F32)
        make_identity(nc, identity)
        ones_row_bf = consts.tile([1, 128], BF16)
        nc.gpsimd.memset(ones_row_bf, 1.0)
        ones_col_bf = consts.tile([128, 1], BF16)
        nc.gpsimd.memset(ones_col_bf, 1.0)
        wcq_f32 = consts.tile([128, 1], F32)
        neg6e4 = consts.tile([128, 1], F32)
        nc.gpsimd.memset(neg6e4, -60000.0)


        def emit_load_head(b):
            L = {}
            qmix_sb = bp.tile(
                [128, QW + 3], BF16, tag="qmix", name=f"qmix_{b}"
            )
            nc.sync.dma_start(out=qmix_sb, in_=qmix_d[b])
            L.update(qryT=qmix_sb[:, 0:128], qry=qmix_sb[:, 128:256],
                     cmT=qmix_sb[:, 256 : 256 + NT],
                     qm=qmix_sb[:, 256 + NT : 256 + NT + 1],
                     wc=qmix_sb[:, QW : QW + 1], wq=qmix_sb[:, QW + 1 : QW + 2],
                     wcq=qmix_sb[:, QW + 2 : QW + 3])
            return L

        def emit_load_ctxT_piece(b, L, lo, hi):
            if "ctxT" not in L:
                L["ctxT"] = bp.tile(
                    [128, C_LEN], BF16, tag="ctxT", name=f"ctxT_{b}"
                )
            nc.sync.dma_start(out=L["ctxT"][:, lo:hi], in_=ctxT_d[b][:, lo:hi])

        def emit_load_ctx(b, L):
            ctx_sb = bp.tile([128, NT, H], BF16, tag="ctx", name=f"ctx_{b}")
            nc.sync.dma_start(out=ctx_sb, in_=ctx_d[b])
            L["ctx"] = ctx_sb

        def emit_prelims_a(b, L):
            if b == 0:
                nc.vector.tensor_copy(out=wcq_f32, in_=L["wcq"])
            qTw = bp.tile([128, 128], BF16, tag="qTw", name=f"qTw_{b}")
            nc.vector.tensor_scalar_mul(qTw, L["qryT"], wcq_f32)

            ps_misc = psM.tile([128, 512], F32, tag="misc")
            L["ps_sc"] = ps_misc[:, 0:16]
            L["ps_z"] = ps_misc[:, 16:32]
            L["ps_q2c"] = ps_misc[:, 32:33]
            L["ps_sr"] = ps_misc[0:1, 34:162]
            L["ps_qr"] = ps_misc[0:1, 34:162]
            L["ps_zb"] = ps_misc[0:1, 162:178]

            ps_sq = ps_misc[:, 33:34]
            nc.tensor.matmul(ps_sq, L["qryT"], L["wq"], start=True, stop=True)
            qoff = bp.tile([128, 1], F32, tag="qoff", name=f"qoff_{b}")
            nc.scalar.activation(
                out=qoff, in_=L["qm"], func=IDENT, scale=60000.0, bias=neg6e4
            )
            sqm_col = bp.tile([128, 1], F32, tag="sqm", name=f"sqm_{b}")
            nc.scalar.add(out=sqm_col, in_=ps_sq, add=qoff)
            eT = bp.tile([128, C_LEN], BF16, tag="eT", name=f"eT_{b}")
            L.update(qTw=qTw, sqm=sqm_col, eT=eT)

        def emit_prelims_b(b, L):
            nc.tensor.transpose(L["ps_sr"], L["sqm"], identity)
            srow = bp.tile([1, 128], BF16, tag="srow", name=f"srow_{b}")
            nc.vector.tensor_copy(out=srow, in_=L["ps_sr"])

            cmoff = bp.tile([128, NT], F32, tag="cmoff", name=f"cmoff_{b}")
            nc.gpsimd.tensor_copy(out=cmoff, in_=L["cmT"])
            nc.gpsimd.tensor_scalar(
                out=cmoff, in0=cmoff, scalar1=1.0, scalar2=60000.0,
                op0=mybir.AluOpType.subtract, op1=mybir.AluOpType.mult,
            )

            m_buf = bp.tile([128, NT], F32, tag="m_buf", name=f"m_buf_{b}")
            u_buf = bp.tile([128, NT], F32, tag="u_buf", name=f"u_buf_{b}")
            e_b = bp.tile([128, NT], BF16, tag="e_b", name=f"e_b_{b}")
            gbuf = bp.tile([128, NT, 256], BF16, tag="gbuf", name=f"gbuf_{b}")
            g4buf = bp.tile([128, NT, 128], BF16, tag="g4buf", name=f"g4buf_{b}")
            L.update(srow=srow, cmoff=cmoff,
                     m_buf=m_buf, u_buf=u_buf, e_b=e_b,
                     gbuf=gbuf, g4buf=g4buf)

        def emit_tt(b, L, g):
            c0 = g * GS * 128
            ps_tt = psTT.tile([128, GS * 128], F32, tag="TT")
            nc.tensor.matmul(
                ps_tt, L["qTw"], L["ctxT"][:, c0 : c0 + GS * 128],
                start=True, stop=True,
            )
            eT_g = L["eT"][:, c0 : c0 + GS * 128]
            nc.scalar.activation(out=eT_g, in_=ps_tt, func=EXP, bias=L["sqm"])

        def emit_phase1(b, L, g, halves=False):
            qry, ctx, gbuf = L["qry"], L["ctx"], L["gbuf"]
            ps_cq = psCQ.tile(
                [128, GS, 128], F32, tag="CQ", name=f"CQ_{b}_{g}"
            )
            dr = bp.tile([128, GS], F32, tag=f"dr{b}_{g % 2}", name=f"dr{b}_{g}")
            waves = ((0, 2), (2, 4)) if halves else ((0, 4),)
            for lo, hi in waves:
                ws = slice(g * GS + lo, g * GS + hi)
                for i in range(lo, hi):
                    t = g * GS + i
                    eT_t = L["eT"][:, t * 128 : (t + 1) * 128]
                    nc.tensor.matmul(
                        ps_cq[:, i], eT_t, qry, start=True, stop=True
                    )
                    nc.tensor.matmul(
                        L["ps_z"][:, t : t + 1], eT_t, ones_col_bf,
                        start=True, stop=True,
                    )
                nc.vector.reciprocal(out=dr[:, lo:hi], in_=L["ps_z"][:, ws])
                if g % 2 == 0:
                    w = hi - lo
                    dr_b = dr[:, lo:hi].unsqueeze(2).broadcast_to([128, w, 128])
                    nc.vector.tensor_mul(
                        out=gbuf[:, ws, 0:128], in0=ps_cq[:, lo:hi], in1=dr_b
                    )
                else:
                    for i in range(lo, hi):
                        t = g * GS + i
                        nc.scalar.activation(
                            out=gbuf[:, t, 0:128], in_=ps_cq[:, i], func=IDENT,
                            scale=dr[:, i : i + 1],
                        )
                eng = nc.gpsimd if g % 2 == 0 else nc.vector
                eng.tensor_mul(
                    out=gbuf[:, ws, 128:256], in0=ctx[:, ws],
                    in1=gbuf[:, ws, 0:128],
                )
                nc.sync.dma_start(out=g23_d[b][:, ws], in_=gbuf[:, ws])

        def emit_phase2(b, L, g):
            ctxT, qTw, srow = L["ctxT"], L["qTw"], L["srow"]
            ps = slice(g * GS, (g + 1) * GS)
            ps_T = psT.tile([128, GS, 128], F32, tag="T")
            for i in range(GS):
                t = g * GS + i
                ctxT_t = ctxT[:, t * 128 : (t + 1) * 128]
                nc.tensor.matmul(ps_T[:, i], ctxT_t, qTw, start=True, stop=False)
                nc.tensor.matmul(ps_T[:, i], ones_row_bf, srow, start=False, stop=True)
                nc.tensor.matmul(
                    L["ps_sc"][:, t : t + 1], ctxT_t, L["wc"], start=True, stop=True
                )
            nc.vector.reduce_max(out=L["m_buf"][:, ps], in_=ps_T, axis=AX)
            if g % 2 == 1:
                pp = slice((g - 1) * GS, (g + 1) * GS)
                u_g = L["u_buf"][:, pp]
                nc.vector.tensor_add(
                    out=u_g, in0=L["ps_sc"][:, pp], in1=L["m_buf"][:, pp]
                )
                nc.vector.tensor_add(out=u_g, in0=u_g, in1=L["cmoff"][:, pp])
                e_g = L["e_b"][:, pp]
                nc.scalar.activation(out=e_g, in_=u_g, func=EXP)
                nc.tensor.matmul(
                    L["ps_zb"][:, pp], ones_col_bf, e_g, start=True, stop=True
                )

        def emit_q2c_mms(b, L):
            for t in range(NT):
                nc.tensor.matmul(
                    L["ps_q2c"], L["ctx"][:, t], L["e_b"][:, t : t + 1],
                    start=(t == 0), stop=(t == NT - 1),
                )

        def emit_tail(b, L):
            ctx, g4buf, e_b = L["ctx"], L["g4buf"], L["e_b"]
            z_tot = bp.tile([1, 1], F32, tag="z_tot")
            nc.vector.reduce_sum(out=z_tot, in_=L["ps_zb"], axis=AX)
            zr = bp.tile([1, 1], F32, tag="zr")
            nc.vector.reciprocal(out=zr, in_=z_tot)

            q2c_col = bp.tile([128, 1], F32, tag="q2c_col")
            nc.scalar.copy(out=q2c_col, in_=L["ps_q2c"])
            nc.tensor.transpose(L["ps_qr"], q2c_col, identity)
            q2c_row = bp.tile([1, 128], BF16, tag="q2c_row")
            nc.vector.tensor_scalar_mul(q2c_row, L["ps_qr"], zr)

            ps_bc = psTT.tile([128, GS * 128], F32, tag="TT", name=f"bc_{b}")
            nc.tensor.matmul(
                ps_bc[:, 0:128], ones_row_bf, q2c_row, start=True, stop=True
            )
            bc_sb = bp.tile([128, 128], BF16, tag="bc_sb")
            nc.scalar.copy(out=bc_sb, in_=ps_bc[:, 0:128])
            bc8 = bc_sb.unsqueeze(1).broadcast_to([128, 8, 128])
            bc4 = bc_sb.unsqueeze(1).broadcast_to([128, GS, 128])
            g4flat = g4buf.rearrange("p t h -> p (t h)")
            nc.vector.tensor_mul(out=g4buf[:, 0:8], in0=ctx[:, 0:8], in1=bc8)
            nc.gpsimd.tensor_mul(
                out=g4buf[:, 12:16], in0=ctx[:, 12:16], in1=bc4
            )
            nc.sync.dma_start(out=g4_d[b][:, 0:1024], in_=g4flat[:, 0:1024])
            nc.vector.tensor_mul(out=g4buf[:, 8:12], in0=ctx[:, 8:12], in1=bc4)
            nc.sync.dma_start(out=g4_d[b][:, 1024:1536], in_=g4flat[:, 1024:1536])
            nc.sync.dma_start(out=g4_d[b][:, 1536:2048], in_=g4flat[:, 1536:2048])

        Ls = [None] * B_PER_CORE
        Ls[0] = emit_load_head(0)
        emit_load_ctxT_piece(0, Ls[0], 0, 512)
        emit_load_ctxT_piece(0, Ls[0], 512, 1024)
        Ls[1] = emit_load_head(1)
        emit_load_ctxT_piece(0, Ls[0], 1024, 2048)
        emit_load_ctxT_piece(1, Ls[1], 0, 1024)
        emit_load_ctx(0, Ls[0])
        emit_load_ctxT_piece(1, Ls[1], 1024, 2048)
        emit_load_ctx(1, Ls[1])

        with tc.high_priority():
            emit_prelims_a(0, Ls[0])
            emit_prelims_a(1, Ls[1])
            emit_tt(0, Ls[0], 0)
            emit_tt(1, Ls[1], 0)
        emit_prelims_b(0, Ls[0])
        emit_prelims_b(1, Ls[1])
        for g in range(NG):
            if g + 1 < NG:
                emit_tt(0, Ls[0], g + 1)
            emit_phase1(0, Ls[0], g)
            emit_phase2(0, Ls[0], g)
            if g + 1 < NG:
                emit_tt(1, Ls[1], g + 1)
            emit_phase2(1, Ls[1], g)
            emit_phase1(1, Ls[1], g)
        emit_q2c_mms(0, Ls[0])
        emit_q2c_mms(1, Ls[1])
        emit_tail(0, Ls[0])
        emit_tail(1, Ls[1])

    return nc


_NC_CACHE = None


def _get_nc():
    global _NC_CACHE
    if _NC_CACHE is None:
        _NC_CACHE = build_nc()
    return _NC_CACHE


def _prep_core_inputs(context, query, W, context_mask, query_mask, sl):
    import ml_dtypes

    BF = ml_dtypes.bfloat16
    ctx = np.asarray(context[sl], dtype=np.float32)
    qry = np.asarray(query[sl], dtype=np.float32)
    ctx_bf = ctx.astype(BF)
    qry_bf = qry.astype(BF)
    n = ctx.shape[0]
    cmT = (
        np.asarray(context_mask[sl], dtype=np.float32)
        .reshape(n, NT, 128)
        .transpose(0, 2, 1)
        .astype(BF)
    )
    qm = np.asarray(query_mask[sl], dtype=np.float32).astype(BF)[:, :, None]
    w3 = np.stack(
        [
            np.asarray(W[:H], dtype=np.float32),
            np.asarray(W[H : 2 * H], dtype=np.float32),
            np.asarray(W[2 * H :], dtype=np.float32),
        ],
        axis=1,
    ).astype(BF)
    w3b = np.broadcast_to(w3[None], (n, H, 3))
    qmix = np.concatenate(
        [qry_bf.transpose(0, 2, 1), qry_bf, cmT, qm, w3b], axis=2
    )
    return {
        "ctxT": np.ascontiguousarray(ctx_bf.transpose(0, 2, 1)),
        "ctx": np.ascontiguousarray(
            ctx_bf.reshape(n, NT, 128, H).transpose(0, 2, 1, 3)
        ),
        "qmix": np.ascontiguousarray(qmix),
    }


def kernel(context, query, W, context_mask, query_mask):
    from concourse.bass_utils import run_bass_kernel_spmd

    context = np.ascontiguousarray(np.asarray(context, dtype=np.float32))
    query = np.ascontiguousarray(np.asarray(query, dtype=np.float32))
    W = np.ascontiguousarray(np.asarray(W, dtype=np.float32))
    context_mask = np.ascontiguousarray(np.asarray(context_mask, dtype=np.int32))
    query_mask = np.ascontiguousarray(np.asarray(query_mask, dtype=np.int32))

    nc = _get_nc()
    in_maps = []
    for c in range(N_CORES):
        sl = slice(c * B_PER_CORE, (c + 1) * B_PER_CORE)
        in_maps.append(
            _prep_core_inputs(context, query, W, context_mask, query_mask, sl)
        )
    res = run_bass_kernel_spmd(nc, in_maps, core_ids=list(range(N_CORES)))

    out = np.empty((B, C_LEN, 4 * H), dtype=np.float32)
    out[:, :, 0:128] = context
    for c in range(N_CORES):
        sl = slice(c * B_PER_CORE, (c + 1) * B_PER_CORE)
        g23 = np.asarray(res.results[c]["G23"]).astype(np.float32)
        g4 = np.asarray(res.results[c]["G4"]).astype(np.float32)
        out[sl, :, 128:384] = g23.transpose(0, 2, 1, 3).reshape(
            B_PER_CORE, C_LEN, 256
        )
        out[sl, :, 384:512] = (
            g4.reshape(B_PER_CORE, 128, NT, 128)
            .transpose(0, 2, 1, 3)
            .reshape(B_PER_CORE, C_LEN, 128)
        )
    return out


if __name__ == "__main__":
    from concourse.timeline_sim import TimelineSim

    nc = build_nc()
    dur = TimelineSim(nc).simulate()
    print(f"TimelineSim estimated duration: {dur:.0f} ns")


# revision 72
# speedup vs baseline: 1.0173x; 1.0037x over previous
import sys

sys.path.insert(0, "/opt/trn_rl_repo")

import numpy as np

import concourse.bass as bass
import concourse.tile as tile
from concourse import mybir
from concourse.masks import make_identity
from concourse.vector_clock import ScopedClock

B, C_LEN, Q_LEN, H = 16, 2048, 128, 128
N_CORES = 8
B_PER_CORE = B // N_CORES
NT = C_LEN // 128
NG = 4
GS = NT // NG
F32 = mybir.dt.float32
BF16 = mybir.dt.bfloat16
I32 = mybir.dt.int32
AX = mybir.AxisListType.X
EXP = mybir.ActivationFunctionType.Exp
IDENT = mybir.ActivationFunctionType.Identity

MAX_WAITS_PER_INST = 1


def _split_excess_waits(nc, insts):
    out = []
    for inst in insts:
        si = getattr(inst, "sync_info", None)
        waits = list(si.on_wait) if si is not None and si.on_wait else []
        if len(waits) > MAX_WAITS_PER_INST and type(inst).__name__.startswith("Inst"):
            extra = waits[: -MAX_WAITS_PER_INST or None]
            keep = waits[-MAX_WAITS_PER_INST:]
            for i in range(0, len(extra), MAX_WAITS_PER_INST):
                out.append(
                    mybir.InstNoOp(
                        name=nc.get_next_instruction_name(),
                        sync_info=mybir.SyncInfo(
                            on_wait=extra[i : i + MAX_WAITS_PER_INST], on_update=[]
                        ),
                        bass_nofuse=True,
                        engine=inst.engine,
                    )
                )
            inst.sync_info = mybir.SyncInfo(
                on_wait=keep, on_update=list(si.on_update or [])
            )
        out.append(inst)
    return out


class SplitDrainTileContext(tile.TileContext):

    def _lower_ordered_insts(self, ordered):
        for bb_name in list(ordered.keys()):
            ordered[bb_name] = _split_excess_waits(self.nc, ordered[bb_name])
        return super()._lower_ordered_insts(ordered)

    def _drain_and_barrier(self, tick_clock, wait_clock):
        nc = self.nc
        drain_inst = nc.sync.drain()
        wait_clock.add_sem_waits(
            drain_inst.ins, ScopedClock({None: tick_clock.global_clock})
        )
        si = drain_inst.ins.sync_info
        waits = list(si.on_wait) if si is not None and si.on_wait else []
        if waits:
            drain_inst.ins.sync_info = mybir.SyncInfo(
                on_wait=[], on_update=list(si.on_update or [])
            )
            engs = [nc.sync, nc.vector, nc.scalar, nc.tensor, nc.gpsimd]
            for j, i in enumerate(range(0, len(waits), MAX_WAITS_PER_INST)):
                nop = engs[j % len(engs)].nop()
                nop.ins.sync_info = mybir.SyncInfo(
                    on_wait=waits[i : i + MAX_WAITS_PER_INST], on_update=[]
                )
        nc.all_engine_barrier()
        assert self.sems is not None
        popped = nc._tile_sem_poison_stack.pop()
        assert popped is self._sem_poison
        nc.clear_and_free_semaphores(list(self.sems.allocated().values()))
        nc.all_engine_barrier()


def build_nc() -> bass.Bass:
    nc = bass.Bass()
    ctxT_d = nc.dram_tensor("ctxT", [B_PER_CORE, H, C_LEN], BF16, kind="ExternalInput")
    ctx_d = nc.dram_tensor("ctx", [B_PER_CORE, 128, NT, H], BF16, kind="ExternalInput")
    QW = 256 + NT + 1
    qmix_d = nc.dram_tensor(
        "qmix", [B_PER_CORE, 128, QW + 3], BF16, kind="ExternalInput"
    )
    g23_d = nc.dram_tensor("G23", [B_PER_CORE, 128, NT, 256], BF16, kind="ExternalOutput")
    g4_d = nc.dram_tensor("G4", [B_PER_CORE, 128, NT * 128], BF16, kind="ExternalOutput")

    from contextlib import ExitStack

    with SplitDrainTileContext(nc) as tc, ExitStack() as es:
        consts = es.enter_context(tc.tile_pool(name="consts", bufs=1))
        bp = es.enter_context(tc.tile_pool(name="bp", bufs=2))
        psT = es.enter_context(tc.tile_pool(name="psT", bufs=2, space="PSUM"))
        psTT = es.enter_context(tc.tile_pool(name="psTT", bufs=2, space="PSUM"))
        psCQ = es.enter_context(tc.tile_pool(name="psCQ", bufs=2, space="PSUM"))
        psM = es.enter_context(tc.tile_pool(name="psM", bufs=2, space="PSUM"))

        identity = consts.tile([128, 128], F32)
        make_identity(nc, identity)
        ones_row_bf = consts.tile([1, 128], BF16)
        nc.gpsimd.memset(ones_row_bf, 1.0)
        ones_col_bf = consts.tile([128, 1], BF16)
        nc.gpsimd.memset(ones_col_bf, 1.0)
        wcq_f32 = consts.tile([128, 1], F32)
        neg6e4 = consts.tile([128, 1], F32)
        nc.gpsimd.memset(neg6e4, -60000.0)


        def emit_load_head(b):
            L = {}
            qmix_sb = bp.tile(
                [128, QW + 3], BF16, tag="qmix", name=f"qmix_{b}"
            )
            nc.sync.dma_start(out=qmix_sb, in_=qmix_d[b])
            L.update(qryT=qmix_sb[:, 0:128], qry=qmix_sb[:, 128:256],
                     cmT=qmix_sb[:, 256 : 256 + NT],
                     qm=qmix_sb[:, 256 + NT : 256 + NT + 1],
                     wc=qmix_sb[:, QW : QW + 1], wq=qmix_sb[:, QW + 1 : QW + 2],
                     wcq=qmix_sb[:, QW + 2 : QW + 3])
            return L

        def emit_load_ctxT_piece(b, L, lo, hi):
            if "ctxT" not in L:
                L["ctxT"] = bp.tile(
                    [128, C_LEN], BF16, tag="ctxT", name=f"ctxT_{b}"
                )
            nc.sync.dma_start(out=L["ctxT"][:, lo:hi], in_=ctxT_d[b][:, lo:hi])

        def emit_load_ctx(b, L):
            ctx_sb = bp.tile([128, NT, H], BF16, tag="ctx", name=f"ctx_{b}")
            nc.sync.dma_start(out=ctx_sb, in_=ctx_d[b])
            L["ctx"] = ctx_sb

        def emit_prelims_a(b, L):
            if b == 0:
                nc.vector.tensor_copy(out=wcq_f32, in_=L["wcq"])
            qTw = bp.tile([128, 128], BF16, tag="qTw", name=f"qTw_{b}")
            nc.vector.tensor_scalar_mul(qTw, L["qryT"], wcq_f32)

            ps_misc = psM.tile([128, 512], F32, tag="misc")
            L["ps_sc"] = ps_misc[:, 0:16]
            L["ps_z"] = ps_misc[:, 16:32]
            L["ps_q2c"] = ps_misc[:, 32:33]
            L["ps_sr"] = ps_misc[0:1, 34:162]
            L["ps_qr"] = ps_misc[0:1, 34:162]
            L["ps_zb"] = ps_misc[0:1, 162:178]

            ps_sq = ps_misc[:, 33:34]
            nc.tensor.matmul(ps_sq, L["qryT"], L["wq"], start=True, stop=True)
            qoff = bp.tile([128, 1], F32, tag="qoff", name=f"qoff_{b}")
            nc.scalar.activation(
                out=qoff, in_=L["qm"], func=IDENT, scale=60000.0, bias=neg6e4
            )
            sqm_col = bp.tile([128, 1], F32, tag="sqm", name=f"sqm_{b}")
            nc.scalar.add(out=sqm_col, in_=ps_sq, add=qoff)
            eT = bp.tile([128, C_LEN], BF16, tag="eT", name=f"eT_{b}")
            L.update(qTw=qTw, sqm=sqm_col, eT=eT)

        def emit_prelims_b(b, L):
            nc.tensor.transpose(L["ps_sr"], L["sqm"], identity)
            srow = bp.tile([1, 128], BF16, tag="srow", name=f"srow_{b}")
            nc.vector.tensor_copy(out=srow, in_=L["ps_sr"])

            cmoff = bp.tile([128, NT], F32, tag="cmoff", name=f"cmoff_{b}")
            nc.gpsimd.tensor_copy(out=cmoff, in_=L["cmT"])
            nc.gpsimd.tensor_scalar(
                out=cmoff, in0=cmoff, scalar1=1.0, scalar2=60000.0,
                op0=mybir.AluOpType.subtract, op1=mybir.AluOpType.mult,
            )

            m_buf = bp.tile([128, NT], F32, tag="m_buf", name=f"m_buf_{b}")
            u_buf = bp.tile([128, NT], F32, tag="u_buf", name=f"u_buf_{b}")
            e_b = bp.tile([128, NT], BF16, tag="e_b", name=f"e_b_{b}")
            gbuf = bp.tile([128, NT, 256], BF16, tag="gbuf", name=f"gbuf_{b}")
            g4buf = bp.tile([128, NT, 128], BF16, tag="g4buf", name=f"g4buf_{b}")
            L.update(srow=srow, cmoff=cmoff,
                     m_buf=m_buf, u_buf=u_buf, e_b=e_b,
                     gbuf=gbuf, g4buf=g4buf)

        def emit_tt(b, L, g):
            c0 = g * GS * 128
            ps_tt = psTT.tile([128, GS * 128], F32, tag="TT")
            nc.tensor.matmul(
                ps_tt, L["qTw"], L["ctxT"][:, c0 : c0 + GS * 128],
                start=True, stop=True,
            )
            eT_g = L["eT"][:, c0 : c0 + GS * 128]
            nc.scalar.activation(out=eT_g, in_=ps_tt, func=EXP, bias=L["sqm"])

        def emit_phase1(b, L, g, halves=False):
            qry, ctx, gbuf = L["qry"], L["ctx"], L["gbuf"]
            ps_cq = psCQ.tile(
                [128, GS, 128], F32, tag="CQ", name=f"CQ_{b}_{g}"
            )
            dr = bp.tile([128, GS], F32, tag=f"dr{b}_{g % 2}", name=f"dr{b}_{g}")
            waves = ((0, 2), (2, 4)) if halves else ((0, 4),)
            for lo, hi in waves:
                ws = slice(g * GS + lo, g * GS + hi)
                for i in range(lo, hi):
                    t = g * GS + i
                    eT_t = L["eT"][:, t * 128 : (t + 1) * 128]
                    nc.tensor.matmul(
                        L["ps_z"][:, t : t + 1], eT_t, ones_col_bf,
                        start=True, stop=True,
                    )
                nc.vector.reciprocal(out=dr[:, lo:hi], in_=L["ps_z"][:, ws])
                for i in range(lo, hi):
                    t = g * GS + i
                    eT_t = L["eT"][:, t * 128 : (t + 1) * 128]
                    nc.tensor.matmul(
                        ps_cq[:, i], eT_t, qry, start=True, stop=True
                    )
                if g % 2 == 0:
                    w = hi - lo
                    dr_b = dr[:, lo:hi].unsqueeze(2).broadcast_to([128, w, 128])
                    nc.vector.tensor_mul(
                        out=gbuf[:, ws, 0:128], in0=ps_cq[:, lo:hi], in1=dr_b
                    )
                else:
                    for i in range(lo, hi):
                        t = g * GS + i
                        nc.scalar.activation(
                            out=gbuf[:, t, 0:128], in_=ps_cq[:, i], func=IDENT,
                            scale=dr[:, i : i + 1],
                        )
                eng = nc.gpsimd if g % 2 == 0 else nc.vector
                eng.tensor_mul(
                    out=gbuf[:, ws, 128:256], in0=ctx[:, ws],
                    in1=gbuf[:, ws, 0:128],
                )
                nc.sync.dma_start(out=g23_d[b][:, ws], in_=gbuf[:, ws])

        def emit_phase2(b, L, g):
            ctxT, qTw, srow = L["ctxT"], L["qTw"], L["srow"]
            ps = slice(g * GS, (g + 1) * GS)
            ps_T = psT.tile([128, GS, 128], F32, tag="T")
            for i in range(GS):
                t = g * GS + i
                ctxT_t = ctxT[:, t * 128 : (t + 1) * 128]
                nc.tensor.matmul(ps_T[:, i], ctxT_t, qTw, start=True, stop=False)
                nc.tensor.matmul(ps_T[:, i], ones_row_bf, srow, start=False, stop=True)
                nc.tensor.matmul(
                    L["ps_sc"][:, t : t + 1], ctxT_t, L["wc"], start=True, stop=True
                )
            nc.vector.reduce_max(out=L["m_buf"][:, ps], in_=ps_T, axis=AX)
            if g % 2 == 1:
                pp = slice((g - 1) * GS, (g + 1) * GS)
                u_g = L["u_buf"][:, pp]
                nc.vector.tensor_add(
                    out=u_g, in0=L["ps_sc"][:, pp], in1=L["m_buf"][:, pp]
                )
                nc.vector.tensor_add(out=u_g, in0=u_g, in1=L["cmoff"][:, pp])
                e_g = L["e_b"][:, pp]
                nc.scalar.activation(out=e_g, in_=u_g, func=EXP)
                nc.tensor.matmul(
                    L["ps_zb"][:, pp], ones_col_bf, e_g, start=True, stop=True
                )

        def emit_q2c_mms(b, L):
            for t in range(NT):
                nc.tensor.matmul(
                    L["ps_q2c"], L["ctx"][:, t], L["e_b"][:, t : t + 1],
                    start=(t == 0), stop=(t == NT - 1),
                )

        def emit_tail(b, L):
            ctx, g4buf, e_b = L["ctx"], L["g4buf"], L["e_b"]
            z_tot = bp.tile([1, 1], F32, tag="z_tot")
            nc.vector.reduce_sum(out=z_tot, in_=L["ps_zb"], axis=AX)
            zr = bp.tile([1, 1], F32, tag="zr")
            nc.vector.reciprocal(out=zr, in_=z_tot)

            q2c_col = bp.tile([128, 1], F32, tag="q2c_col")
            nc.scalar.copy(out=q2c_col, in_=L["ps_q2c"])
            nc.tensor.transpose(L["ps_qr"], q2c_col, identity)
            q2c_row = bp.tile([1, 128], BF16, tag="q2c_row")
            nc.vector.tensor_scalar_mul(q2c_row, L["ps_qr"], zr)

            ps_bc = psTT.tile([128, GS * 128], F32, tag="TT", name=f"bc_{b}")
            nc.tensor.matmul(
                ps_bc[:, 0:128], ones_row_bf, q2c_row, start=True, stop=True
            )
            bc_sb = bp.tile([128, 128], BF16, tag="bc_sb")
            nc.scalar.copy(out=bc_sb, in_=ps_bc[:, 0:128])
            bc8 = bc_sb.unsqueeze(1).broadcast_to([128, 8, 128])
            bc4 = bc_sb.unsqueeze(1).broadcast_to([128, GS, 128])
            g4flat = g4buf.rearrange("p t h -> p (t h)")
            nc.vector.tensor_mul(out=g4buf[:, 0:8], in0=ctx[:, 0:8], in1=bc8)
            nc.gpsimd.tensor_mul(
                out=g4buf[:, 12:16], in0=ctx[:, 12:16], in1=bc4
            )
            nc.sync.dma_start(out=g4_d[b][:, 0:1024], in_=g4flat[:, 0:1024])
            nc.vector.tensor_mul(out=g4buf[:, 8:12], in0=ctx[:, 8:12], in1=bc4)
            nc.sync.dma_start(out=g4_d[b][:, 1024:1536], in_=g4flat[:, 1024:1536])
            nc.sync.dma_start(out=g4_d[b][:, 1536:2048], in_=g4flat[:, 1536:2048])

        Ls = [None] * B_PER_CORE
        Ls[0] = emit_load_head(0)
        emit_load_ctxT_piece(0, Ls[0], 0, 512)
        emit_load_ctxT_piece(0, Ls[0], 512, 1024)
        Ls[1] = emit_load_head(1)
        emit_load_ctxT_piece(0, Ls[0], 1024, 2048)
        emit_load_ctxT_piece(1, Ls[1], 0, 1024)
        emit_load_ctx(0, Ls[0])
        emit_load_ctxT_piece(1, Ls[1], 1024, 2048)
        emit_load_ctx(1, Ls[1])

        with tc.high_priority():
            emit_prelims_a(0, Ls[0])
            emit_prelims_a(1, Ls[1])
            emit_tt(0, Ls[0], 0)
            emit_tt(1, Ls[1], 0)
        emit_prelims_b(0, Ls[0])
        emit_prelims_b(1, Ls[1])
        for g in range(NG):
            if g + 1 < NG:
                emit_tt(0, Ls[0], g + 1)
            emit_phase1(0, Ls[0], g)
            emit_phase2(0, Ls[0], g)
            if g + 1 < NG:
                emit_tt(1, Ls[1], g + 1)
            emit_phase2(1, Ls[1], g)
            emit_phase1(1, Ls[1], g)
        emit_q2c_mms(0, Ls[0])
        emit_q2c_mms(1, Ls[1])
        emit_tail(0, Ls[0])
        emit_tail(1, Ls[1])

    return nc


_NC_CACHE = None


def _get_nc():
    global _NC_CACHE
    if _NC_CACHE is None:
        _NC_CACHE = build_nc()
    return _NC_CACHE


def _prep_core_inputs(context, query, W, context_mask, query_mask, sl):
    import ml_dtypes

    BF = ml_dtypes.bfloat16
    ctx = np.asarray(context[sl], dtype=np.float32)
    qry = np.asarray(query[sl], dtype=np.float32)
    ctx_bf = ctx.astype(BF)
    qry_bf = qry.astype(BF)
    n = ctx.shape[0]
    cmT = (
        np.asarray(context_mask[sl], dtype=np.float32)
        .reshape(n, NT, 128)
        .transpose(0, 2, 1)
        .astype(BF)
    )
    qm = np.asarray(query_mask[sl], dtype=np.float32).astype(BF)[:, :, None]
    w3 = np.stack(
        [
            np.asarray(W[:H], dtype=np.float32),
            np.asarray(W[H : 2 * H], dtype=np.float32),
            np.asarray(W[2 * H :], dtype=np.float32),
        ],
        axis=1,
    ).astype(BF)
    w3b = np.broadcast_to(w3[None], (n, H, 3))
    qmix = np.concatenate(
        [qry_bf.transpose(0, 2, 1), qry_bf, cmT, qm, w3b], axis=2
    )
    return {
        "ctxT": np.ascontiguousarray(ctx_bf.transpose(0, 2, 1)),
        "ctx": np.ascontiguousarray(
            ctx_bf.reshape(n, NT, 128, H).transpose(0, 2, 1, 3)
        ),
        "qmix": np.ascontiguousarray(qmix),
    }


def kernel(context, query, W, context_mask, query_mask):
    from concourse.bass_utils import run_bass_kernel_spmd

    context = np.ascontiguousarray(np.asarray(context, dtype=np.float32))
    query = np.ascontiguousarray(np.asarray(query, dtype=np.float32))
    W = np.ascontiguousarray(np.asarray(W, dtype=np.float32))
    context_mask = np.ascontiguousarray(np.asarray(context_mask, dtype=np.int32))
    query_mask = np.ascontiguousarray(np.asarray(query_mask, dtype=np.int32))

    nc = _get_nc()
    in_maps = []
    for c in range(N_CORES):
        sl = slice(c * B_PER_CORE, (c + 1) * B_PER_CORE)
        in_maps.append(
            _prep_core_inputs(context, query, W, context_mask, query_mask, sl)
        )
    res = run_bass_kernel_spmd(nc, in_maps, core_ids=list(range(N_CORES)))

    out = np.empty((B, C_LEN, 4 * H), dtype=np.float32)
    out[:, :, 0:128] = context
    for c in range(N_CORES):
        sl = slice(c * B_PER_CORE, (c + 1) * B_PER_CORE)
        g23 = np.asarray(res.results[c]["G23"]).astype(np.float32)
        g4 = np.asarray(res.results[c]["G4"]).astype(np.float32)
        out[sl, :, 128:384] = g23.transpose(0, 2, 1, 3).reshape(
            B_PER_CORE, C_LEN, 256
        )
        out[sl, :, 384:512] = (
            g4.reshape(B_PER_CORE, 128, NT, 128)
            .transpose(0, 2, 1, 3)
            .reshape(B_PER_CORE, C_LEN, 128)
        )
    return out


if __name__ == "__main__":
    from concourse.timeline_sim import TimelineSim

    nc = build_nc()
    dur = TimelineSim(nc).simulate()
    print(f"TimelineSim estimated duration: {dur:.0f} ns")
